# revision 31
# baseline (speedup 1.0000x reference)
"""Causal single-head attention (S=4096, D=1024, fp32) on 8 TRN2 NeuronCores.

v9: causal fold-balanced schedule, fp8-transport sharded K/V projection,
DoubleRow fp8 scores AND A@V.

Row ownership (fold): core c owns row blocks c and 15-c (256 rows each),
packed as qT columns [top | bot]. Key block k is needed for the top half iff
k <= c and for the bot half iff k <= 15-c, so the uniform SPMD program runs
key blocks 0-7 against all 512 rows and blocks 8-15 against the bot 256
only; per-core causal variation lives in small 0/1 mask tiles multiplied
into p. Score work is 24/64 of the dense rectangle.

K/V projection is sharded 8-way (core c computes blocks c and 8+c) and
distributed by three fp8 AllGathers, ordered by when consumers need them:
  G1: K blocks 0-7   G2: V blocks 0-7 + K blocks 8-15   G3: V blocks 8-15
The first collective can't execute before a ~70-80us cross-core launch
barrier (axon environment floor), and the CC stream is serial (~90us for
8MB), so blocks 0,1 are made fully gather-free: every core computes K AND V
for key blocks 0,1 locally in bf16 and runs their scores + A@V while the
gathers are still in flight. (Local bf16 V for early blocks is also a
precision requirement: rows 0-511 average too few keys to tolerate fp8 V.)

Numerics: q,k fp8 via DoubleRow (2x PE); p is bf16 for blocks 0,1 and fp8
for blocks >= 2 (rows there average >= 513 keys, washing out fp8 p/V noise)
which enables DoubleRow A@V against fp8 V. exp uses bias -2 to keep p in
e4m3 normal range (cancels in softmax). 1/sqrt(D) is folded as D**-0.25
into BOTH W_q and W_k so fp8 q/k stay in e4m3 normal range.
"""

import numpy as np
import ml_dtypes

import concourse.bacc as bacc
import concourse.tile as tile
from concourse import mybir
from concourse.bass_utils import run_bass_kernel_spmd

S = 4096
D = 1024
NCORES = 8
P = 128
RPC = 512          # rows per core
KB = 256           # key block
DC = 8             # d_in chunks of 128
BF = mybir.dt.bfloat16
F8 = mybir.dt.float8e4
F32 = mybir.dt.float32
EXP = mybir.ActivationFunctionType.Exp
DR = mybir.MatmulPerfMode.DoubleRow
DEBUG = False

bf16 = ml_dtypes.bfloat16
f8e4 = ml_dtypes.float8_e4m3fn

# K sections: sec = ohi (d_out chunk; pairs (2i, 2i+1) feed DoubleRow),
#             offset = key within the 256-key block. Partition = d0.
# V sections: sec = 4*half + 2*kt + s, offset = d % 256
#             (d = 512*half + 256*s + offset). Partition = key within tile kt.
#             For fixed half, (kt, s, off) is contiguous -> DoubleRow rhs.


def build_nc():
    nc = bacc.Bacc(None, target_bir_lowering=False, debug=False)

    xq = nc.declare_dram_parameter("xqt", [D, RPC], BF, isOutput=False)
    xk = nc.declare_dram_parameter("xkt", [D, 512], BF, isOutput=False)
    xv = nc.declare_dram_parameter("xvt", [D, 512], BF, isOutput=False)
    xk01 = nc.declare_dram_parameter("xk01t", [D, 512], BF, isOutput=False)
    xv01 = nc.declare_dram_parameter("xv01t", [D, 512], BF, isOutput=False)
    wq = nc.declare_dram_parameter("wqt", [D, D], BF, isOutput=False)
    wk = nc.declare_dram_parameter("wkt", [D, D], BF, isOutput=False)
    wv = nc.declare_dram_parameter("wvt", [D, D], BF, isOutput=False)
    mlo = nc.declare_dram_parameter("mlo", [8, P, 2, 512], BF, isOutput=False)
    mhi = nc.declare_dram_parameter("mhi", [8, P, 2, 256], BF, isOutput=False)
    out = nc.declare_dram_parameter("out", [RPC, D], F32, isOutput=True)

    kvin_kk = nc.dram_tensor("kvin_kk", [P, 16, 256], F8)
    kvout_kk = nc.dram_tensor("kvout_kk", [NCORES * P, 16, 256], F8)
    kvin_v0 = nc.dram_tensor("kvin_v0", [P, 8, 256], F8)
    kvout_v0 = nc.dram_tensor("kvout_v0", [NCORES * P, 8, 256], F8)
    kvin_v1 = nc.dram_tensor("kvin_v1", [P, 8, 256], F8)
    kvout_v1 = nc.dram_tensor("kvout_v1", [NCORES * P, 8, 256], F8)
    if DEBUG:
        dbg_sums = nc.declare_dram_parameter("dbg_sums", [P, 64], F32, isOutput=True)

    with tile.TileContext(nc) as tc:
        with (
            tc.tile_pool(name="persist", bufs=1) as persist,
            tc.tile_pool(name="wp", bufs=1) as wp,
            tc.tile_pool(name="stg", bufs=1) as stg,
            tc.tile_pool(name="kvs", bufs=3) as kvs,
            tc.tile_pool(name="vbs", bufs=1) as vbs,
            tc.tile_pool(name="pbs", bufs=1) as pbs,
            tc.tile_pool(name="op", bufs=4) as op,
            tc.tile_pool(name="pps", bufs=3, space="PSUM") as pps,
            tc.tile_pool(name="avs", bufs=2, space="PSUM") as avs,
            tc.tile_pool(name="ops", bufs=1, space="PSUM") as ops,
        ):
            ones = persist.tile([P, 16], BF, tag="ones", name="ones")
            nc.vector.memset(ones[:], 1.0)
            nbias = persist.tile([P, 1], F32, tag="nbias", name="nbias")
            nc.vector.memset(nbias[:], -2.0)
            ones_f = persist.tile([P, 16], F32, tag="ones_f", name="ones_f")
            nc.vector.memset(ones_f[:], 1.0)
            qT = persist.tile([P, 4, 2, RPC], F8, tag="qT", name="qT")
            acc = {}
            for st in range(4):
                acc[st] = persist.tile([P, D], F32, tag=f"acc{st}", name=f"acc{st}")
                nc.vector.memset(acc[st][:], 0.0)
            vloc = [persist.tile([P, 2, 2, 2, 256], BF, tag=f"vloc{b}", name=f"vloc{b}")
                    for b in range(2)]
            kloc = [persist.tile([P, 8, 256], F8, tag=f"kloc{b}", name=f"kloc{b}")
                    for b in range(2)]
            mlo_t = [persist.tile([P, 2, 512], BF, tag=f"mlo{k}", name=f"mlo{k}") for k in range(8)]
            mhi_t = [persist.tile([P, 2, 256], BF, tag=f"mhi{k}", name=f"mhi{k}") for k in range(8)]
            sums = ops.tile([P, 64], F32, tag="sums", name="sums")
            cs_lo = persist.tile([P, 512], F32, tag="cs_lo", name="cs_lo")
            nc.vector.memset(cs_lo[:], 0.0)
            cs_hi = persist.tile([P, 256], F32, tag="cs_hi", name="cs_hi")
            nc.vector.memset(cs_hi[:], 0.0)

            # ---- input loads on sync (ordered by first use) ----
            wk_t = [wp.tile([P, D], BF, tag=f"wk{d}", name=f"wk{d}") for d in range(DC)]
            wv_t = [wp.tile([P, D], BF, tag=f"wv{d}", name=f"wv{d}") for d in range(DC)]
            wq_t = [wp.tile([P, D], BF, tag=f"wq{d}", name=f"wq{d}") for d in range(DC)]
            xk_t = [wp.tile([P, 512], BF, tag=f"xk{d}", name=f"xk{d}") for d in range(DC)]
            xv_t = [wp.tile([P, 512], BF, tag=f"xv{d}", name=f"xv{d}") for d in range(DC)]
            xq_t = [wp.tile([P, RPC], BF, tag=f"xq{d}", name=f"xq{d}") for d in range(DC)]
            xk01_t = [wp.tile([P, 512], BF, tag=f"xk01{d}", name=f"xk01{d}") for d in range(DC)]
            xv01_t = [wp.tile([P, 512], BF, tag=f"xv01{d}", name=f"xv01{d}") for d in range(DC)]
            for d in range(DC):
                r = slice(d * P, (d + 1) * P)
                nc.sync.dma_start(out=wk_t[d][:], in_=wk[r, :])
                nc.sync.dma_start(out=xk_t[d][:], in_=xk[r, :])
            for d in range(DC):
                r = slice(d * P, (d + 1) * P)
                nc.sync.dma_start(out=wv_t[d][:], in_=wv[r, :])
                nc.sync.dma_start(out=xv_t[d][:], in_=xv[r, :])
            for d in range(DC):
                r = slice(d * P, (d + 1) * P)
                nc.sync.dma_start(out=xk01_t[d][:], in_=xk01[r, :])
                nc.sync.dma_start(out=xv01_t[d][:], in_=xv01[r, :])
                nc.sync.dma_start(out=wq_t[d][:], in_=wq[r, :])
                nc.sync.dma_start(out=xq_t[d][:], in_=xq[r, :])
            for k in range(8):
                nc.sync.dma_start(out=mlo_t[k][:], in_=mlo[k, :, :, :])
            for k in range(8):
                nc.sync.dma_start(out=mhi_t[k][:], in_=mhi[k, :, :, :])

            def proj_k(xt, cols, put):
                # K^T proj of 256 keys; put(ohi, psum[:, 0:256]) consumes
                for ohi in range(DC):
                    ps = pps.tile([P, 512], F32, tag="pp", name="ppk")
                    for d in range(DC):
                        nc.tensor.matmul(
                            ps[:, 0:256],
                            lhsT=wk_t[d][:, ohi * P:(ohi + 1) * P],
                            rhs=xt[d][:, cols],
                            start=(d == 0),
                            stop=(d == DC - 1),
                        )
                    put(ohi, ps)

            def proj_v(xt, base, put):
                # V proj of 256 keys; put(kt, half, s, psum[:, s*256:...])
                for kt in range(2):
                    for half in range(2):
                        ps = pps.tile([P, 512], F32, tag="pp", name="ppv")
                        for d in range(DC):
                            nc.tensor.matmul(
                                ps[:],
                                lhsT=xt[d][:, base + kt * P:base + (kt + 1) * P],
                                rhs=wv_t[d][:, half * 512:(half + 1) * 512],
                                start=(d == 0),
                                stop=(d == DC - 1),
                            )
                        for s in range(2):
                            put(kt, half, s, ps)

            def gather(kvi, kvo):
                nc.gpsimd.collective_compute(
                    "AllGather",
                    mybir.AluOpType.bypass,
                    replica_groups=[[0, 1, 2, 3, 4, 5, 6, 7]],
                    ins=[kvi[:].opt()],
                    outs=[kvo[:].opt()],
                )

            # kvin staging DMAs on scalar; doorbells (gpsimd) fire early.
            # K for ALL blocks goes first so both score superblocks unblock
            # as soon as the first gather lands; V follows in consumption
            # order so only the last A@V chains trail the final gather.
            sg1 = stg.tile([P, 16, 256], F8, tag="sg1", name="sg1")
            proj_k(xk_t, slice(0, 256),
                   lambda ohi, ps: nc.scalar.copy(sg1[:, ohi, :], ps[:, 0:256]))
            proj_k(xk_t, slice(256, 512),
                   lambda ohi, ps: nc.scalar.copy(sg1[:, 8 + ohi, :], ps[:, 0:256]))
            nc.scalar.dma_start(out=kvin_kk[:], in_=sg1[:])
            gather(kvin_kk, kvout_kk)

            sg2 = stg.tile([P, 8, 256], F8, tag="sg2", name="sg2")
            proj_v(xv_t, 0,
                   lambda kt, half, s, ps: nc.scalar.copy(
                       sg2[:, 4 * half + 2 * kt + s, :], ps[:, s * 256:(s + 1) * 256]))
            nc.scalar.dma_start(out=kvin_v0[:], in_=sg2[:])
            gather(kvin_v0, kvout_v0)

            sg3 = stg.tile([P, 8, 256], F8, tag="sg3", name="sg3")
            proj_v(xv_t, 256,
                   lambda kt, half, s, ps: nc.scalar.copy(
                       sg3[:, 4 * half + 2 * kt + s, :], ps[:, s * 256:(s + 1) * 256]))
            nc.scalar.dma_start(out=kvin_v1[:], in_=sg3[:])
            gather(kvin_v1, kvout_v1)

            # local K and V for key blocks 0,1: gather-free early work
            for b in range(2):
                proj_k(xk01_t, slice(256 * b, 256 * b + 256),
                       lambda ohi, ps, b=b: nc.scalar.copy(kloc[b][:, ohi, :], ps[:, 0:256]))
                proj_v(xv01_t, 256 * b,
                       lambda kt, half, s, ps, b=b: nc.scalar.copy(
                           vloc[b][:, half, kt, s, :], ps[:, s * 256:(s + 1) * 256]))

            # q projection -> qT fp8 [d0, pair, t, row]
            for ohi in range(DC):
                ps = pps.tile([P, 512], F32, tag="pp", name="ppq")
                for d in range(DC):
                    nc.tensor.matmul(
                        ps[:],
                        lhsT=wq_t[d][:, ohi * P:(ohi + 1) * P],
                        rhs=xq_t[d][:],
                        start=(d == 0),
                        stop=(d == DC - 1),
                    )
                nc.scalar.copy(qT[:, ohi // 2, ohi % 2, :], ps[:])

            # ---- attention ----
            def attn_block(sb, b8):
                blk = 8 * sb + b8
                W = 512 if sb == 0 else 256
                roff = 0 if sb == 0 else 256
                rows = slice(b8 * P, (b8 + 1) * P)
                local = sb == 0 and b8 < 2
                if local:
                    kblk = kloc[b8]
                    vblk = vloc[b8]
                else:
                    kblk = kvs.tile([P, 8, 256], F8, tag="kb", name="kb")
                    if sb == 0:
                        nc.sync.dma_start(out=kblk[:], in_=kvout_kk[rows, 0:8, :])
                    else:
                        nc.sync.dma_start(out=kblk[:], in_=kvout_kk[rows, 8:16, :])
                    # vblk on gpsimd (idle after doorbells): keeps a DMA that
                    # waits a late gather from head-of-line blocking kblk
                    vblk = vbs.tile([P, 2, 2, 2, 256], F8, tag=f"vb{b8}", name=f"vb{b8}")
                    if sb == 0:
                        nc.gpsimd.dma_start(out=vblk[:], in_=kvout_v0[rows, 0:8, :])
                    else:
                        nc.gpsimd.dma_start(out=vblk[:], in_=kvout_v1[rows, 0:8, :])
                pdt = BF if local else F8
                pblk = pbs.tile([P, 2, 512], pdt, tag=f"pb{'l' if local else 'g'}{b8}",
                                name=f"pb{b8}")
                mt = mlo_t[b8] if sb == 0 else mhi_t[b8]
                for kt in range(2):
                    sp = pps.tile([P, 512], F32, tag="pp", name="sp")
                    for i in range(4):
                        nc.tensor.matmul(
                            sp[:, 0:W],
                            lhsT=kblk[:, 2 * i:2 * i + 2, kt * P:(kt + 1) * P],
                            rhs=qT[:, i, :, roff:roff + W],
                            start=(i == 0),
                            stop=(i == 3),
                            perf_mode=DR,
                        )
                    nc.scalar.activation(pblk[:, kt, 0:W], sp[:, 0:W], EXP, bias=nbias[:])
                    nc.vector.tensor_mul(pblk[:, kt, 0:W], pblk[:, kt, 0:W], mt[:, kt, 0:W])
                    # denominator partials accumulate elementwise on vector;
                    # a handful of ones-matmuls fold the partition axis later
                    cs = cs_lo if sb == 0 else cs_hi
                    nc.vector.tensor_add(cs[:, 0:W], cs[:, 0:W], pblk[:, kt, 0:W])
                return pblk, vblk

            def fold_sums(cs, stls, first, stop_regs):
                # sums[region] += ones-matmul over partition axis of colsum.
                # start=True clears the WHOLE psum bank: first call only.
                for j, (stl, reg) in enumerate(stls):
                    nc.tensor.matmul(
                        sums[:, reg * 16:(reg + 1) * 16],
                        lhsT=cs[:, stl * P:(stl + 1) * P],
                        rhs=ones_f[:],
                        start=(first and j == 0),
                        stop=(reg in stop_regs),
                        skip_group_check=True,
                    )

            def attn_av(sb, tiles, local):
                # one psum chain per (row subtile, d half) over this tile set
                sts = (0, 1, 2, 3) if sb == 0 else (2, 3)
                roff = 0 if sb == 0 else 256
                for st in sts:
                    stl = st * P - roff
                    for half in range(2):
                        av = avs.tile([P, 512], F32, tag="av", name="av")
                        n = len(tiles)
                        for j, (pblk, vblk) in enumerate(tiles):
                            if local:
                                for kt in range(2):
                                    nc.tensor.matmul(
                                        av[:],
                                        lhsT=pblk[:, kt, stl:stl + P],
                                        rhs=vblk[:, half, kt, :, :],
                                        start=(j == 0 and kt == 0),
                                        stop=(j == n - 1 and kt == 1),
                                    )
                            else:
                                nc.tensor.matmul(
                                    av[:],
                                    lhsT=pblk[:, :, stl:stl + P],
                                    rhs=vblk[:, half, :, :, :],
                                    start=(j == 0),
                                    stop=(j == n - 1),
                                    perf_mode=DR,
                                )
                        nc.vector.tensor_add(
                            acc[st][:, half * 512:(half + 1) * 512],
                            acc[st][:, half * 512:(half + 1) * 512],
                            av[:],
                        )

            def finalize(st):
                ssb = op.tile([P, 1], F32, tag="ssb", name="ssb")
                nc.vector.tensor_copy(ssb[:], sums[:, st * 16:st * 16 + 1])
                rec = op.tile([P, 1], F32, tag=f"rec{st}", name=f"rec{st}")
                nc.vector.reciprocal(rec[:], ssb[:])
                for half in range(2):
                    osb = op.tile([P, 512], F32, tag="osb", name="osb")
                    nc.vector.tensor_scalar_mul(osb[:], acc[st][:, half * 512:(half + 1) * 512], rec[:])
                    nc.sync.dma_start(out=out[st * P:(st + 1) * P, half * 512:(half + 1) * 512], in_=osb[:])

            # blocks 0,1 are fully local: scores + A@V before any gather lands
            tiles01 = [attn_block(0, b8) for b8 in range(2)]
            attn_av(0, tiles01, local=True)
            tiles27 = [attn_block(0, b8) for b8 in range(2, 8)]
            attn_av(0, tiles27, local=False)
            fold_sums(cs_lo, [(0, 0), (1, 1), (2, 2), (3, 3)], first=True,
                      stop_regs={0, 1})
            finalize(0)
            finalize(1)
            tiles1 = [attn_block(1, b8) for b8 in range(8)]
            attn_av(1, tiles1, local=False)
            fold_sums(cs_hi, [(0, 2), (1, 3)], first=False, stop_regs={2, 3})
            if DEBUG:
                dsm = persist.tile([P, 64], F32, tag="dsm", name="dsm")
                nc.vector.tensor_copy(dsm[:], sums[:])
                nc.sync.dma_start(out=dbg_sums[:], in_=dsm[:])
            finalize(2)
            finalize(3)
    return nc


_CACHE = {}


def _get_nc():
    if "nc" not in _CACHE:
        nc = build_nc()
        nc.compile()
        _CACHE["nc"] = nc
    return _CACHE["nc"]


def build_in_maps(inputs):
    x_q = np.asarray(inputs["encodings_for_q"], dtype=np.float32)
    x_k = np.asarray(inputs["encodings_for_k"], dtype=np.float32)
    x_v = np.asarray(inputs["encodings_for_v"], dtype=np.float32)
    W_q = np.asarray(inputs["W_q"], dtype=np.float32)
    W_k = np.asarray(inputs["W_k"], dtype=np.float32)
    W_v = np.asarray(inputs["W_v"], dtype=np.float32)

    qs = D ** -0.25
    wqt = np.ascontiguousarray(W_q.T * qs).astype(bf16)
    wkt = np.ascontiguousarray(W_k.T * qs).astype(bf16)
    wvt = np.ascontiguousarray(W_v.T).astype(bf16)
    xk01t = np.ascontiguousarray(x_k[0:512].T).astype(bf16)
    xv01t = np.ascontiguousarray(x_v[0:512].T).astype(bf16)

    in_maps = []
    for c in range(NCORES):
        top = slice(KB * c, KB * (c + 1))
        bot = slice(KB * (15 - c), KB * (16 - c))
        xqt = np.ascontiguousarray(
            np.concatenate([x_q[top], x_q[bot]], axis=0).T).astype(bf16)
        ksel = np.concatenate([x_k[top], x_k[KB * (8 + c):KB * (9 + c)]], axis=0)
        vsel = np.concatenate([x_v[top], x_v[KB * (8 + c):KB * (9 + c)]], axis=0)
        xkt = np.ascontiguousarray(ksel.T).astype(bf16)
        xvt = np.ascontiguousarray(vsel.T).astype(bf16)

        rows = np.concatenate([np.arange(KB * c, KB * (c + 1)),
                               np.arange(KB * (15 - c), KB * (16 - c))])
        p_idx = np.arange(P)
        mlo = np.zeros((8, P, 2, 512), dtype=np.float32)
        mhi = np.zeros((8, P, 2, 256), dtype=np.float32)
        for k in range(8):
            for t in range(2):
                keys = KB * k + P * t + p_idx
                mlo[k, :, t, :] = (rows[None, :] >= keys[:, None])
                keys_h = 2048 + KB * k + P * t + p_idx
                mhi[k, :, t, :] = (rows[None, 256:] >= keys_h[:, None])
        in_maps.append(
            dict(
                xqt=xqt, xkt=xkt, xvt=xvt, xk01t=xk01t, xv01t=xv01t,
                wqt=wqt, wkt=wkt, wvt=wvt,
                mlo=mlo.astype(bf16), mhi=mhi.astype(bf16),
            )
        )
    return in_maps


def kernel(**inputs):
    nc = _get_nc()
    in_maps = build_in_maps(inputs)
    res = run_bass_kernel_spmd(nc, in_maps, list(range(NCORES)))
    outs = [np.asarray(res.results[i]["out"], dtype=np.float32) for i in range(NCORES)]
    full = np.empty((S, D), dtype=np.float32)
    for c in range(NCORES):
        full[KB * c:KB * (c + 1)] = outs[c][0:KB]
        full[KB * (15 - c):KB * (16 - c)] = outs[c][KB:2 * KB]
    return full
